# revision 30
# baseline (speedup 1.0000x reference)
"""Trainium2 Bass kernel for a pre-LN transformer encoder layer.

Contract: kernel(**inputs) takes the FULL inputs (x [1,4096,1024] plus
weights/biases) and returns the FULL output [1,4096,1024].

Sharding: sequence-parallel over 8 NeuronCores; each core owns 512 rows.

v2 design (fp8 + DoubleRow):
- h, Q, K, V, exp(scores), ctx all fp8 e4m3 (verified rel-err ~0.002).
- QKV/O projections and ctx use fp8 DoubleRow matmuls (2x MAC rate:
  256-deep contraction at 1 cycle/row).
- Scores are output-rate-bound (128 out/cycle) so fp8 plain matmuls
  (same speed as bf16, half the SBUF/D2D bytes).
- exp split between ACT (true exp -> fp8) and DVE (int8 bit-trick
  2^x approx, ~1.6% err, 335ns vs 612ns per [128,512] tile).
- KV AllGather in fp8, chunked per head-quad, overlapped w/ attention.
- FFN kept bf16 (fp8 there costs ~1.9e-2 rel err vs the 2e-2 gate).
"""

import numpy as np
from contextlib import ExitStack

import concourse.bass as bass
import concourse.mybir as mybir
import concourse.tile as tile
from concourse import bacc
from concourse.bass_utils import run_bass_kernel_spmd
from concourse.masks import make_identity

P = 128
NCORES = 8
S = 4096
SL = S // NCORES          # 512 local rows
D = 1024
H = 16
DK = D // H               # 64
F = 4096
EPS = 1e-6

F32 = mybir.dt.float32
BF16 = mybir.dt.bfloat16
F8 = mybir.dt.float8e4
I8 = mybir.dt.int8
AF = mybir.ActivationFunctionType
OP = mybir.AluOpType
DR = mybir.MatmulPerfMode.DoubleRow

KN = P * SL               # K chunk elems per (hh, core): [128 dk, 512 keys]
VN = SL * P               # V chunk elems per (hh, core): [512 keys, 128 dims]
CH = KN + VN              # elems (fp8 bytes) per head-pair per core

LOG2E_8 = 8 * 1.4426950408889634
EXPBIAS8 = 56.0           # 7 * 8 (e4m3 bias in units of 1/8 exponent)

_CACHE = {}


def _build(ln1_a, ln1_b, ln2_a, ln2_b):
    nc = bacc.Bacc("TRN2", target_bir_lowering=False, debug=False,
                   num_devices=NCORES)

    x_d = nc.dram_tensor("x_loc", [SL, D], F32, kind="ExternalInput")
    # stacked cc-pair fp8 weights: [4, 128, 2, 1024] viewed as [4,128,2048]
    wq_d = nc.dram_tensor("Wq_stk", [4, P, 2 * D], F8, kind="ExternalInput")
    wk_d = nc.dram_tensor("Wk_stk", [4, P, 2 * D], F8, kind="ExternalInput")
    wv_d = nc.dram_tensor("Wv_stk", [4, P, 2 * D], F8, kind="ExternalInput")
    wo_d = nc.dram_tensor("Wo_stk", [4, P, 2 * D], F8, kind="ExternalInput")
    w1_d = nc.dram_tensor("W1", [D, F], BF16, kind="ExternalInput")
    w2_d = nc.dram_tensor("W2", [F, D], BF16, kind="ExternalInput")
    bq_d = nc.dram_tensor("bq", [D], F32, kind="ExternalInput")
    bk_d = nc.dram_tensor("bk", [D], F32, kind="ExternalInput")
    b1_d = nc.dram_tensor("b1", [F], F32, kind="ExternalInput")
    bx3_d = nc.dram_tensor("bx3", [3, D], BF16, kind="ExternalInput")
    y_d = nc.dram_tensor("y_loc", [SL, D], F32, kind="ExternalOutput")

    with tile.TileContext(nc) as tc, ExitStack() as ctx:
        const = ctx.enter_context(tc.tile_pool(name="const", bufs=1))
        stat = ctx.enter_context(tc.tile_pool(name="stat", bufs=16))
        tmp = ctx.enter_context(tc.tile_pool(name="tmp", bufs=2))
        dram = ctx.enter_context(tc.tile_pool(name="dram", bufs=1, space="DRAM"))

        # ---------------- constants ----------------
        identb = const.tile([P, P], BF16)
        make_identity(nc, identb)
        ones65 = const.tile([65, P], BF16)
        nc.vector.memset(ones65[:], 1.0)
        ones1 = ones65[0:1, :]
        heat_a = const.tile([P, P], BF16)
        nc.vector.memset(heat_a[:], 0.5)
        heat_r = const.tile([P, SL], BF16)
        nc.vector.memset(heat_r[:], 0.5)

        def heat_burst(ps_pool, n, nm):
            """dense matmul burst to hold the PE p-state at full clock."""
            hp = ps_pool.tile([P, SL], F32, name=f"heat_{nm}", tag="heat")
            for i in range(n):
                nc.tensor.matmul(hp[:], heat_a[:], heat_r[:],
                                 start=True, stop=True)

        bq_t = const.tile([P, 8], F32)
        nc.sync.dma_start(bq_t[:], bq_d.rearrange("(c p) -> p c", p=P))
        bq8 = const.tile([P, 8], F32)
        nc.vector.tensor_scalar(bq8[:], bq_t[:], 0.125, None, OP.mult)
        bk_t = const.tile([P, 8], F32)
        nc.sync.dma_start(bk_t[:], bk_d.rearrange("(c p) -> p c", p=P))
        b1_t = const.tile([P, 32], F32)
        nc.sync.dma_start(b1_t[:], b1_d.rearrange("(c p) -> p c", p=P))

        rcon = const.tile([65, D], BF16)
        nc.sync.dma_start(rcon[0:1, :], bx3_d[0:1, :])
        nc.sync.dma_start(rcon[32:33, :], bx3_d[1:2, :])
        nc.sync.dma_start(rcon[64:65, :], bx3_d[2:3, :])
        bvr = rcon[0:1, :]
        bor = rcon[32:33, :]
        b2r = rcon[64:65, :]

        def layer_norm_to_T(src_big, a_val, b_val, hT, tp_psum,
                            burst_pool=None):
            """src_big [P, 4, D] F32 -> hT [P, 8, SL] (transposed LN).

            Internally bf16 (fp8 PE-transpose needs stride-2 outputs); the
            psum->SBUF copy converts to hT's dtype for free."""
            for j in range(4):
                st = stat.tile([P, 2, 6], F32, name=f"bst{j}", tag="bst")
                for hhalf in range(2):
                    nc.vector.bn_stats(
                        st[:, hhalf, :],
                        src_big[:, j, hhalf * 512:(hhalf + 1) * 512])
                mv = stat.tile([P, 2], F32, name=f"mv{j}", tag="mv")
                nc.vector.bn_aggr(mv[:], st[:])
                rr = stat.tile([P, 1], F32, name=f"rr{j}", tag="rr")
                nc.scalar.activation(rr[:], mv[:, 1:2], AF.Sqrt,
                                     scale=float(D) / (D - 1))
                nc.vector.tensor_scalar_add(rr[:], rr[:], EPS)
                nc.vector.reciprocal(rr[:], rr[:])
                nc.vector.tensor_scalar_mul(rr[:], rr[:], float(a_val))
                bp = stat.tile([P, 1], F32, name=f"bp{j}", tag="bp")
                nc.vector.tensor_tensor(bp[:], mv[:, 0:1], rr[:], OP.mult)
                nc.vector.tensor_scalar(bp[:], bp[:], -1.0, float(b_val),
                                        OP.mult, OP.add)
                h = tmp.tile([P, D], BF16, name=f"h{j}", tag="h")
                nc.scalar.activation(h[:], src_big[:, j, :], AF.Identity,
                                     bias=bp[:], scale=rr[:])
                if burst_pool is not None and j in (0, 2):
                    heat_burst(burst_pool, 10, f"ln{j}")
                for half in range(2):
                    tp = tp_psum.tile([P, 512], BF16, name=f"tp{j}_{half}",
                                      tag="tp")
                    for k in range(4):
                        cc = half * 4 + k
                        nc.tensor.transpose(tp[:, k * P:(k + 1) * P],
                                            h[:, cc * P:(cc + 1) * P],
                                            identb[:])
                    nc.vector.tensor_copy(
                        hT[:, half * 4:(half + 1) * 4, j * P:(j + 1) * P],
                        tp.rearrange("p (c q) -> p c q", q=P))

        groups = [list(range(NCORES))]
        # tiny dummy AllGather issued first: absorbs the ~50us one-time
        # collective-engine startup latency so the real KV gathers are prompt
        warm_in = dram.tile([256], F8, name="warm_in")
        warm_out = dram.tile([NCORES * 256], F8, name="warm_out",
                             addr_space="Shared")
        warm_src = const.tile([1, 256], F8)
        nc.vector.memset(warm_src[:], 0.0)
        nc.sync.dma_start(warm_in.rearrange("(p n) -> p n", p=1),
                          warm_src[:])
        nc.gpsimd.collective_compute(
            "AllGather", OP.bypass, replica_groups=groups,
            ins=[warm_in.opt()], outs=[warm_out.opt()])

        # staggered chunks: first gathers start early to hide D2D latency
        CHUNKS = [(0,), (1,), (2, 3), (4, 5), (6, 7)]
        CHUNK_OF = {hh: gi for gi, hhs in enumerate(CHUNKS) for hh in hhs}
        CHUNK_OFF = {hh: hhs.index(hh) for hhs in CHUNKS for hh in hhs}
        KVCs = [dram.tile([len(hhs) * CH], F8, name=f"kvc{gi}")
                for gi, hhs in enumerate(CHUNKS)]
        GKVs = [dram.tile([NCORES * len(hhs) * CH], F8, name=f"gkv{gi}",
                          addr_space="Shared") for gi, hhs in enumerate(CHUNKS)]

        # W1/W2 stream pools (bf16): DMAs prefetch during attention.
        w1pool = ctx.enter_context(tc.tile_pool(name="w1pool", bufs=32))
        w2pool = ctx.enter_context(tc.tile_pool(name="w2pool", bufs=28))
        x2_pool = ctx.enter_context(tc.tile_pool(name="x2_pool", bufs=1))
        hb_pool = ctx.enter_context(tc.tile_pool(name="hb_pool", bufs=1))

        with (
            tc.tile_pool(name="x_pool", bufs=1) as x_pool,
            tc.tile_pool(name="ctx_pool", bufs=1) as ctx_pool,
        ):
            x_big = x_pool.tile([P, 4, D], F32)
            with tc.tile_pool(name="qt_pool", bufs=1) as qt_pool:
                # fp8 Q tiles; the passive halves are zero (memset once)
                QTA = qt_pool.tile([P, 8, SL], F8, name="QTA")
                QTB = qt_pool.tile([P, 8, SL], F8, name="QTB")
                nc.vector.memset(QTA[64:128, :, :], 0.0)
                nc.vector.memset(QTB[0:64, :, :], 0.0)

                # ---------------- phase 1: LN1 + transpose ----------------
                with tc.tile_pool(name="hT_pool", bufs=1) as hT_pool:
                    hT = hT_pool.tile([P, 8, SL], F8)
                    with tc.tile_pool(name="tp1", bufs=2, space="PSUM") as tpp:
                        for j in range(4):
                            for hf in range(2):
                                cs = slice(hf * 512, (hf + 1) * 512)
                                nc.sync.dma_start(
                                    x_big[:, j, cs],
                                    x_d[j * P:(j + 1) * P, cs])
                        layer_norm_to_T(x_big, ln1_a, ln1_b, hT, tpp,
                                        burst_pool=tpp)
                        heat_burst(tpp, 6, "p2")

                    # ------- phase 2: hh-major K/V/Q + chunked gathers -----
                    with (
                        tc.tile_pool(name="wbig", bufs=13) as wbig,
                        tc.tile_pool(name="kvstage", bufs=2) as kvstage,
                        tc.tile_pool(name="qkps", bufs=2, space="PSUM") as qkps,
                    ):
                        wkt, wvt, wqt = [], [], []
                        for nm, wd, lst in (("wk", wk_d, wkt),
                                            ("wv", wv_d, wvt),
                                            ("wq", wq_d, wqt)):
                            for cp in range(4):
                                w = wbig.tile([P, 2, D], F8, name=f"{nm}{cp}",
                                              tag="wbig")
                                nc.sync.dma_start(
                                    w.rearrange("p a b -> p (a b)"),
                                    wd[cp])
                                lst.append(w)

                        for hh in range(8):
                            hs = slice(hh * P, (hh + 1) * P)
                            # K chunk: [128 dk, 512 keys] fp8
                            ps = qkps.tile([P, SL], F32, name=f"kps{hh}",
                                           tag="qk")
                            for cp in range(4):
                                nc.tensor.matmul(
                                    ps[:], wkt[cp][:, :, hs],
                                    hT[:, 2 * cp:2 * cp + 2, :],
                                    start=(cp == 0), stop=(cp == 3),
                                    perf_mode=DR)
                            kstg = kvstage.tile([P, SL], F8,
                                                name=f"kstg{hh}", tag="kstg")
                            nc.scalar.activation(kstg[:], ps[:], AF.Identity,
                                                 bias=bk_t[:, hh:hh + 1])
                            cb = KVCs[CHUNK_OF[hh]][CHUNK_OFF[hh] * CH:]
                            nc.sync.dma_start(
                                cb[0:KN].rearrange("(d q) -> d q", q=SL),
                                kstg[:])
                            # V chunk: [512 keys, 128 dims] fp8
                            vstg = kvstage.tile([P, 4, P], F8,
                                                name=f"vstg{hh}", tag="vstg")
                            for sb in range(4):
                                psv = qkps.tile([P, P], F32,
                                                name=f"vps{hh}_{sb}", tag="qk")
                                for cp in range(4):
                                    nc.tensor.matmul(
                                        psv[:],
                                        hT[:, 2 * cp:2 * cp + 2,
                                           sb * P:(sb + 1) * P],
                                        wvt[cp][:, :, hs],
                                        start=(cp == 0), stop=False,
                                        perf_mode=DR)
                                nc.tensor.matmul(psv[:], ones1[:], bvr[:, hs],
                                                 start=False, stop=True)
                                nc.scalar.copy(vstg[:, sb, :], psv[:])
                            nc.sync.dma_start(
                                cb[KN:CH].rearrange(
                                    "(sb p e) -> p sb e", p=P, e=P),
                                vstg[:])
                            # Q chunk -> fp8 QTA/QTB active halves
                            psq = qkps.tile([P, SL], F32, name=f"qps{hh}",
                                            tag="qk")
                            for cp in range(4):
                                nc.tensor.matmul(
                                    psq[:], wqt[cp][:, :, hs],
                                    hT[:, 2 * cp:2 * cp + 2, :],
                                    start=(cp == 0), stop=(cp == 3),
                                    perf_mode=DR)
                            nc.scalar.activation(
                                QTA[0:64, hh, :], psq[0:64, :], AF.Identity,
                                bias=bq8[0:64, hh:hh + 1], scale=1.0 / 8.0)
                            nc.scalar.activation(
                                QTB[64:128, hh, :], psq[64:128, :],
                                AF.Identity, bias=bq8[64:128, hh:hh + 1],
                                scale=1.0 / 8.0)
                            gi = CHUNK_OF[hh]
                            if CHUNK_OFF[hh] == len(CHUNKS[gi]) - 1:
                                nc.gpsimd.collective_compute(
                                    "AllGather", OP.bypass,
                                    replica_groups=groups,
                                    ins=[KVCs[gi].opt()],
                                    outs=[GKVs[gi].opt()])

                # Wo (fp8) + W1 + W2 prefetch: DMAs are threaded through the
                # attention steps so they don't jam the KV-gather D2D traffic
                wot = []
                w1t = [[None] * 8 for _ in range(4)]
                w2ts_all = [None] * 32
                prefetch_q = []

                def _pf_wo(cp):
                    w = w1pool.tile([P, 2, D], F8, name=f"wo{cp}", tag="w1")
                    nc.sync.dma_start(w.rearrange("p a b -> p (a b)"),
                                      wo_d[cp])
                    wot.append(w)

                def _pf_w1(qq, cc):
                    w = w1pool.tile([P, F // 4], BF16,
                                    name=f"w1_{qq}_{cc}", tag="w1")
                    nc.sync.dma_start(
                        w[:], w1_d[cc * P:(cc + 1) * P,
                                   qq * 1024:(qq + 1) * 1024])
                    w1t[qq][cc] = w

                def _pf_w2(fg):
                    w2t = w2pool.tile([P, D], BF16, name=f"w2_{fg}",
                                      tag="w2")
                    nc.sync.dma_start(w2t[:], w2_d[fg * P:(fg + 1) * P, :])
                    w2ts_all[fg] = w2t

                for cp in range(4):
                    prefetch_q.append((lambda cp=cp: _pf_wo(cp)))
                for qq in range(2):
                    for cc in range(8):
                        prefetch_q.append((lambda q=qq, c=cc: _pf_w1(q, c)))
                for fg in range(16):
                    prefetch_q.append((lambda f=fg: _pf_w2(f)))
                for qq in range(2, 4):
                    for cc in range(8):
                        prefetch_q.append((lambda q=qq, c=cc: _pf_w1(q, c)))
                for fg in range(16, 32):
                    prefetch_q.append((lambda f=fg: _pf_w2(f)))

                # ---------------- phase 4: attention ----------------
                ctxT = ctx_pool.tile([P, 8, SL], F8)
                with (
                    tc.tile_pool(name="kst", bufs=8) as kst,
                    tc.tile_pool(name="vst", bufs=1) as vst,
                    tc.tile_pool(name="esb", bufs=6) as esb,
                    tc.tile_pool(name="bcs_pool", bufs=2) as bcs_pool,
                    tc.tile_pool(name="rs_pool", bufs=1) as rs_pool,
                    tc.tile_pool(name="spsum", bufs=2, space="PSUM") as spsum,
                    tc.tile_pool(name="cpsum", bufs=2, space="PSUM") as cpsum,
                ):
                    # rotating fp8 V buffers with pre-set ones columns
                    vt_bufs = [vst.tile([P, 4, 2 * P], F8, name=f"vtb{i}")
                               for i in range(4)]
                    for vb in vt_bufs:
                        nc.vector.memset(vb[:], 0.0)
                        nc.vector.memset(vb[:, :, 64], 1.0)
                        nc.vector.memset(vb[:, :, 192], 1.0)

                    cps_all = {}

                    def get_cps(hh, i):
                        key = (hh, i)
                        if key not in cps_all:
                            cps_all[key] = cpsum.tile(
                                [P, SL], F32, name=f"ctx{hh}_{i}",
                                tag=f"ctx{i}")
                        return cps_all[key]

                    kts = {}
                    vt4s = {}

                    def load_kv(hh, c):
                        kt = kst.tile([P, SL], F8, name=f"kt{hh}_{c}",
                                      tag="kt")
                        gi = CHUNK_OF[hh]
                        csz = len(CHUNKS[gi]) * CH
                        base = c * csz + CHUNK_OFF[hh] * CH
                        ksec = GKVs[gi][base: base + KN].rearrange(
                            "(d q) -> d q", q=SL)
                        nc.sync.dma_start(kt[:], ksec)
                        kts[(hh, c)] = kt
                        vb = vt_bufs[(hh * NCORES + c) % 4]
                        vsec = GKVs[gi][base + KN: base + CH].rearrange(
                            "(s e) -> s e", e=P)
                        nc.sync.dma_start(
                            vb[:, :, 0:64],
                            vsec[:, 0:64].rearrange("(kbl p) e -> p kbl e",
                                                    p=P))
                        nc.sync.dma_start(
                            vb[:, :, 128:192],
                            vsec[:, 64:128].rearrange("(kbl p) e -> p kbl e",
                                                      p=P))
                        vt4s[(hh, c)] = vb

                    steps = [(hh, c, h01, g)
                             for hh in range(8)
                             for c in range(NCORES)
                             for h01 in range(2)
                             for g in range(2)]

                    def emit_scores_exp(step, si):
                        hh, c, h01, g = step
                        if (hh, c) not in kts:
                            load_kv(hh, c)
                        kt = kts[(hh, c)]
                        rhs_q = (QTA if h01 == 0 else QTB)[:, hh, :]
                        # separate single-bank psum tiles per half so each is
                        # freed independently by its one exp reader
                        ak = si % 2
                        dk = 1 - ak
                        sps = [None, None]
                        for kk in (dk, ak):
                            sps[kk] = spsum.tile(
                                [P, 512], F32,
                                name=f"sp{hh}_{c}_{h01}_{g}_{kk}",
                                tag=f"sp{kk}")
                            kbl = g * 2 + kk
                            nc.tensor.matmul(
                                sps[kk][:], kt[:, kbl * P:(kbl + 1) * P],
                                rhs_q, start=True, stop=True)
                        et = esb.tile([P, 2, 512], F8,
                                      name=f"e{hh}_{c}_{h01}_{g}", tag="et")
                        # both engines in parallel: DVE bit-trick on the half
                        # computed first, ACT true-exp on the other
                        nc.vector.tensor_scalar(
                            et[:, dk, :].bitcast(I8), sps[dk][:],
                            LOG2E_8, EXPBIAS8, OP.mult, OP.add)
                        nc.scalar.activation(et[:, ak, :], sps[ak][:],
                                             AF.Exp)
                        return (step, et)

                    def emit_ctx(item):
                        (hh, c, h01, g), et = item
                        vb = vt4s[(hh, c)]
                        nc.tensor.matmul(
                            get_cps(hh, h01)[:],
                            vb[:, 2 * g:2 * g + 2, h01 * P:(h01 + 1) * P],
                            et[:],
                            start=(c == 0 and g == 0),
                            stop=(c == 7 and g == 1),
                            perf_mode=DR)

                    bcs_of = {}
                    lnt_of = {}

                    def emit_norm_a(hh):
                        # stage 1: denominator copies + Ln; emitted only once
                        # the ctx psum stop has executed (no queue blocking)
                        cps = [cps_all[(hh, 0)], cps_all[(hh, 1)]]
                        den = rs_pool.tile([33, SL], F32, name=f"den{hh}",
                                           tag="den")
                        nc.scalar.copy(den[0:1, :], cps[0][64:65, :])
                        nc.scalar.copy(den[32:33, :], cps[1][64:65, :])
                        # 1/x as exp(-ln x) on ACT: keeps the slow DVE
                        # reciprocal out of the in-order DVE exp stream
                        lnt = rs_pool.tile([33, SL], F32, name=f"lnt{hh}",
                                           tag="lnt")
                        nc.scalar.activation(lnt[:], den[:], AF.Ln)
                        lnt_of[hh] = lnt

                    def emit_norm_a2(hh):
                        # stage 2: Exp + broadcast staging (spreads ACT load)
                        lnt = lnt_of.pop(hh)
                        rcf = rs_pool.tile([33, SL], F32, name=f"rcf{hh}",
                                           tag="rcf")
                        nc.scalar.activation(rcf[:], lnt[:], AF.Exp,
                                             scale=-1.0)
                        # partition_broadcast requires in AND out at
                        # partition base 0 -> separate base-0 tiles
                        rcb = rs_pool.tile([1, SL], F32, name=f"rcb{hh}",
                                           tag="rcb")
                        nc.scalar.copy(rcb[:], rcf[32:33, :])
                        bcsa = bcs_pool.tile([64, SL], F32, name=f"bcsa{hh}",
                                             tag="bcsa")
                        bcsb = bcs_pool.tile([64, SL], F32, name=f"bcsb{hh}",
                                             tag="bcsb")
                        nc.gpsimd.partition_broadcast(bcsa[:], rcf[0:1, :])
                        nc.gpsimd.partition_broadcast(bcsb[:], rcb[0:1, :])
                        bcs_of[hh] = (bcsa, bcsb)

                    def emit_norm_b(hh):
                        # the two DVE mults, emitted after broadcasts landed
                        cps = [cps_all.pop((hh, 0)), cps_all.pop((hh, 1))]
                        bcsa, bcsb = bcs_of.pop(hh)
                        nc.vector.tensor_tensor(ctxT[0:64, hh, :],
                                                cps[0][0:64, :],
                                                bcsa[:], OP.mult)
                        nc.vector.tensor_tensor(ctxT[64:128, hh, :],
                                                cps[1][0:64, :],
                                                bcsb[:], OP.mult)

                    norm_qa = []
                    norm_qa2 = []
                    norm_qb = []
                    pend = []
                    for si, step in enumerate(steps):
                        if si + 6 < len(steps):
                            nhh, ncc, _, _ = steps[si + 6]
                            if (nhh, ncc) not in kts:
                                load_kv(nhh, ncc)
                        if si >= 40 and prefetch_q and si % 3 == 0:
                            prefetch_q.pop(0)()
                        pend.append(emit_scores_exp(step, si))
                        if len(pend) > 3:
                            it = pend.pop(0)
                            emit_ctx(it)
                            phh, pc, ph01, pg = it[0]
                            if pc == 7 and ph01 == 1 and pg == 1:
                                norm_qa.append((phh, si + 6))
                        if norm_qa and si >= norm_qa[0][1]:
                            hh_n = norm_qa.pop(0)[0]
                            emit_norm_a(hh_n)
                            norm_qa2.append((hh_n, si + 3))
                        if norm_qa2 and si >= norm_qa2[0][1]:
                            hh_n = norm_qa2.pop(0)[0]
                            emit_norm_a2(hh_n)
                            norm_qb.append((hh_n, si + 3))
                        if norm_qb and si >= norm_qb[0][1]:
                            emit_norm_b(norm_qb.pop(0)[0])
                    while prefetch_q:
                        prefetch_q.pop(0)()
                    for it in pend:
                        emit_ctx(it)
                        phh, pc, ph01, pg = it[0]
                        if pc == 7 and ph01 == 1 and pg == 1:
                            norm_qa.append((phh, 0))
                    for hh, _ in norm_qa:
                        emit_norm_a(hh)
                        norm_qa2.append((hh, 0))
                    for hh, _ in norm_qa2:
                        emit_norm_a2(hh)
                        norm_qb.append((hh, 0))
                    for hh, _ in norm_qb:
                        emit_norm_b(hh)

            # ---------------- phase 5: out-proj + residual ----------------
            x2 = x2_pool.tile([P, 4, D], BF16)
            with (
                tc.tile_pool(name="ops", bufs=2, space="PSUM") as opps,
                tc.tile_pool(name="hps5", bufs=1, space="PSUM") as hps5,
            ):
                heat_burst(hps5, 10, "oproj")
                for sb in range(4):
                    for eb in range(2):
                        ps = opps.tile([P, 512], F32, name=f"op{sb}_{eb}",
                                       tag="op")
                        for cp in range(4):
                            nc.tensor.matmul(
                                ps[:],
                                ctxT[:, 2 * cp:2 * cp + 2,
                                     sb * P:(sb + 1) * P],
                                wot[cp][:, :, eb * 512:(eb + 1) * 512],
                                start=(cp == 0), stop=False, perf_mode=DR)
                        nc.tensor.matmul(ps[:], ones65[32:33, :],
                                         bor[:, eb * 512:(eb + 1) * 512],
                                         start=False, stop=True)
                        nc.vector.tensor_tensor(
                            x2[:, sb, eb * 512:(eb + 1) * 512], ps[:],
                            x_big[:, sb, eb * 512:(eb + 1) * 512], OP.add)

        # ---------------- phase 6: LN2 + transpose (bf16) ----------------
        with tc.tile_pool(name="h2T_pool", bufs=1) as h2T_pool:
            h2T = h2T_pool.tile([P, 8, SL], BF16)
            with tc.tile_pool(name="tp2", bufs=2, space="PSUM") as tpp2:
                layer_norm_to_T(x2, ln2_a, ln2_b, h2T, tpp2,
                                burst_pool=tpp2)

            # ------------- phases 7/8: FFN in two halves (bf16) -----------
            with (
                tc.tile_pool(name="atpool", bufs=4) as atpool,
                tc.tile_pool(name="o2ppool", bufs=1) as o2ppool,
                tc.tile_pool(name="outpool", bufs=3) as outpool,
                tc.tile_pool(name="f1ps", bufs=2, space="PSUM") as f1ps,
                tc.tile_pool(name="f2ps", bufs=4, space="PSUM") as f2ps,
            ):
                o2p = o2ppool.tile([P, 4, D], F32)
                with tc.tile_pool(name="hps7", bufs=1, space="PSUM") as hps7:
                    heat_burst(hps7, 10, "ffn")
                for half in range(2):
                    at_h = []
                    for qq in range(half * 2, half * 2 + 2):
                        ATq = atpool.tile([P, 8, SL], BF16,
                                          name=f"at{qq}", tag="at")
                        for fc in range(8):
                            fg = qq * 8 + fc
                            ps = f1ps.tile([P, SL], F32, name=f"f1_{fg}",
                                           tag="f1")
                            for cc in range(8):
                                nc.tensor.matmul(
                                    ps[:],
                                    w1t[qq][cc][:, fc * P:(fc + 1) * P],
                                    h2T[:, cc, :], start=(cc == 0),
                                    stop=(cc == 7))
                            nc.vector.tensor_scalar(ATq[:, fc, :], ps[:],
                                                    b1_t[:, fg:fg + 1],
                                                    0.0, OP.add, OP.max)
                        at_h.append(ATq)
                    w2ts = w2ts_all[half * 16:half * 16 + 16]
                    for eb in range(2):
                        sl = slice(eb * 512, (eb + 1) * 512)
                        pss = [f2ps.tile([P, 512], F32,
                                         name=f"f2_{half}_{eb}_{sb}",
                                         tag="f2") for sb in range(4)]
                        for fcl in range(16):
                            qq, fc = divmod(fcl, 8)
                            for sb in range(4):
                                nc.tensor.matmul(
                                    pss[sb][:],
                                    at_h[qq][:, fc, sb * P:(sb + 1) * P],
                                    w2ts[fcl][:, sl],
                                    start=(fcl == 0),
                                    stop=(half == 0 and fcl == 15))
                        for sb in range(4):
                            ps = pss[sb]
                            if half == 0:
                                nc.vector.tensor_tensor(
                                    o2p[:, sb, sl], ps[:], x2[:, sb, sl],
                                    OP.add)
                            else:
                                nc.tensor.matmul(ps[:], ones65[64:65, :],
                                                 b2r[:, sl],
                                                 start=False, stop=True)
                                ot = outpool.tile([P, 512], F32,
                                                  name=f"ot{sb}_{eb}",
                                                  tag="ot")
                                nc.vector.tensor_tensor(ot[:], ps[:],
                                                        o2p[:, sb, sl],
                                                        OP.add)
                                nc.sync.dma_start(
                                    y_d[sb * P:(sb + 1) * P, sl], ot[:])

    nc.compile()
    return nc


def _stack_pairs(w):
    """[D, N] -> [4, 128, 2*N] with [cp, p, j*N:(j+1)*N] = w[cp*256+j*128+p]."""
    Dd, N = w.shape
    return np.ascontiguousarray(
        w.reshape(4, 2, P, N).transpose(0, 2, 1, 3).reshape(4, P, 2 * N))


def make_in_maps(inp):
    import ml_dtypes
    BF = ml_dtypes.bfloat16
    E4 = ml_dtypes.float8_e4m3
    xf = inp["x"].reshape(S, D)
    shared = {
        "Wq_stk": _stack_pairs(inp["Wq"].astype(E4)),
        "Wk_stk": _stack_pairs(inp["Wk"].astype(E4)),
        "Wv_stk": _stack_pairs(inp["Wv"].astype(E4)),
        "Wo_stk": _stack_pairs(inp["Wo"].astype(E4)),
        "W1": inp["W1"].astype(BF), "W2": inp["W2"].astype(BF),
        "bq": inp["bq"], "bk": inp["bk"], "b1": inp["b1"],
        "bx3": np.stack([inp["bv"], inp["bo"], inp["b2"]]).astype(BF),
    }
    in_maps = []
    for c in range(NCORES):
        m = dict(shared)
        m["x_loc"] = np.ascontiguousarray(xf[c * SL:(c + 1) * SL, :])
        in_maps.append(m)
    return in_maps


def kernel(**inputs):
    inp = {k: np.asarray(v, dtype=np.float32) for k, v in inputs.items()}
    x = inp["x"]
    B = x.shape[0]
    key = (float(inp["ln1_a"][0]), float(inp["ln1_b"][0]),
           float(inp["ln2_a"][0]), float(inp["ln2_b"][0]))
    if key not in _CACHE:
        _CACHE[key] = _build(*key)
    nc = _CACHE[key]

    res = run_bass_kernel_spmd(nc, make_in_maps(inp), list(range(NCORES)))
    out = np.concatenate([res.results[c]["y_loc"] for c in range(NCORES)],
                         axis=0)
    return out.reshape(B, S, D)


# revision 32
# speedup vs baseline: 1.0479x; 1.0479x over previous
"""Trainium2 Bass kernel for a pre-LN transformer encoder layer.

Contract: kernel(**inputs) takes the FULL inputs (x [1,4096,1024] plus
weights/biases) and returns the FULL output [1,4096,1024].

Sharding: sequence-parallel over 8 NeuronCores; each core owns 512 rows.

v2 design (fp8 + DoubleRow):
- h, Q, K, V, exp(scores), ctx all fp8 e4m3 (verified rel-err ~0.002).
- QKV/O projections and ctx use fp8 DoubleRow matmuls (2x MAC rate:
  256-deep contraction at 1 cycle/row).
- Scores are output-rate-bound (128 out/cycle) so fp8 plain matmuls
  (same speed as bf16, half the SBUF/D2D bytes).
- exp split between ACT (true exp -> fp8) and DVE (int8 bit-trick
  2^x approx, ~1.6% err, 335ns vs 612ns per [128,512] tile).
- KV AllGather in fp8, chunked per head-quad, overlapped w/ attention.
- FFN kept bf16 (fp8 there costs ~1.9e-2 rel err vs the 2e-2 gate).
"""

import numpy as np
from contextlib import ExitStack

import concourse.bass as bass
import concourse.mybir as mybir
import concourse.tile as tile
from concourse import bacc
from concourse.bass_utils import run_bass_kernel_spmd
from concourse.masks import make_identity

P = 128
NCORES = 8
S = 4096
SL = S // NCORES          # 512 local rows
D = 1024
H = 16
DK = D // H               # 64
F = 4096
EPS = 1e-6

F32 = mybir.dt.float32
BF16 = mybir.dt.bfloat16
F8 = mybir.dt.float8e4
I8 = mybir.dt.int8
AF = mybir.ActivationFunctionType
OP = mybir.AluOpType
DR = mybir.MatmulPerfMode.DoubleRow

KN = P * SL               # K chunk elems per (hh, core): [128 dk, 512 keys]
VN = SL * P               # V chunk elems per (hh, core): [512 keys, 128 dims]
CH = KN + VN              # elems (fp8 bytes) per head-pair per core

LOG2E_8 = 8 * 1.4426950408889634
EXPBIAS8 = 56.0           # 7 * 8 (e4m3 bias in units of 1/8 exponent)

_CACHE = {}


def _build(ln1_a, ln1_b, ln2_a, ln2_b):
    nc = bacc.Bacc("TRN2", target_bir_lowering=False, debug=False,
                   num_devices=NCORES)

    x_d = nc.dram_tensor("x_loc", [SL, D], F32, kind="ExternalInput")
    # stacked cc-pair fp8 weights: [4, 128, 2, 1024] viewed as [4,128,2048]
    wq_d = nc.dram_tensor("Wq_stk", [4, P, 2 * D], F8, kind="ExternalInput")
    wk_d = nc.dram_tensor("Wk_stk", [4, P, 2 * D], F8, kind="ExternalInput")
    wv_d = nc.dram_tensor("Wv_stk", [4, P, 2 * D], F8, kind="ExternalInput")
    wo_d = nc.dram_tensor("Wo_stk", [4, P, 2 * D], F8, kind="ExternalInput")
    w1_d = nc.dram_tensor("W1", [D, F], BF16, kind="ExternalInput")
    w2_d = nc.dram_tensor("W2", [F, D], BF16, kind="ExternalInput")
    bq_d = nc.dram_tensor("bq", [D], F32, kind="ExternalInput")
    bk_d = nc.dram_tensor("bk", [D], F32, kind="ExternalInput")
    b1_d = nc.dram_tensor("b1", [F], F32, kind="ExternalInput")
    bx3_d = nc.dram_tensor("bx3", [3, D], BF16, kind="ExternalInput")
    y_d = nc.dram_tensor("y_loc", [SL, D], F32, kind="ExternalOutput")

    with tile.TileContext(nc) as tc, ExitStack() as ctx:
        const = ctx.enter_context(tc.tile_pool(name="const", bufs=1))
        stat = ctx.enter_context(tc.tile_pool(name="stat", bufs=16))
        tmp = ctx.enter_context(tc.tile_pool(name="tmp", bufs=2))
        dram = ctx.enter_context(tc.tile_pool(name="dram", bufs=1, space="DRAM"))

        # ---------------- constants ----------------
        identb = const.tile([P, P], BF16)
        make_identity(nc, identb)
        ones65 = const.tile([65, P], BF16)
        nc.vector.memset(ones65[:], 1.0)
        ones1 = ones65[0:1, :]
        heat_a = const.tile([P, P], BF16)
        nc.vector.memset(heat_a[:], 0.5)
        heat_r = const.tile([P, SL], BF16)
        nc.vector.memset(heat_r[:], 0.5)

        def heat_burst(ps_pool, n, nm):
            """dense matmul burst to hold the PE p-state at full clock."""
            hp = ps_pool.tile([P, SL], F32, name=f"heat_{nm}", tag="heat")
            for i in range(n):
                nc.tensor.matmul(hp[:], heat_a[:], heat_r[:],
                                 start=True, stop=True)

        bq_t = const.tile([P, 8], F32)
        nc.sync.dma_start(bq_t[:], bq_d.rearrange("(c p) -> p c", p=P))
        bq8 = const.tile([P, 8], F32)
        nc.vector.tensor_scalar(bq8[:], bq_t[:], 0.125, None, OP.mult)
        bk_t = const.tile([P, 8], F32)
        nc.sync.dma_start(bk_t[:], bk_d.rearrange("(c p) -> p c", p=P))
        b1_t = const.tile([P, 32], F32)
        nc.sync.dma_start(b1_t[:], b1_d.rearrange("(c p) -> p c", p=P))

        rcon = const.tile([65, D], BF16)
        nc.sync.dma_start(rcon[0:1, :], bx3_d[0:1, :])
        nc.sync.dma_start(rcon[32:33, :], bx3_d[1:2, :])
        nc.sync.dma_start(rcon[64:65, :], bx3_d[2:3, :])
        bvr = rcon[0:1, :]
        bor = rcon[32:33, :]
        b2r = rcon[64:65, :]

        def layer_norm_to_T(src_big, a_val, b_val, hT, tp_psum,
                            burst_pool=None):
            """src_big [P, 4, D] F32 -> hT [P, 8, SL] (transposed LN).

            Internally bf16 (fp8 PE-transpose needs stride-2 outputs); the
            psum->SBUF copy converts to hT's dtype for free."""
            for j in range(4):
                st = stat.tile([P, 2, 6], F32, name=f"bst{j}", tag="bst")
                for hhalf in range(2):
                    nc.vector.bn_stats(
                        st[:, hhalf, :],
                        src_big[:, j, hhalf * 512:(hhalf + 1) * 512])
                mv = stat.tile([P, 2], F32, name=f"mv{j}", tag="mv")
                nc.vector.bn_aggr(mv[:], st[:])
                rr = stat.tile([P, 1], F32, name=f"rr{j}", tag="rr")
                nc.scalar.activation(rr[:], mv[:, 1:2], AF.Sqrt,
                                     scale=float(D) / (D - 1))
                nc.vector.tensor_scalar_add(rr[:], rr[:], EPS)
                nc.vector.reciprocal(rr[:], rr[:])
                nc.vector.tensor_scalar_mul(rr[:], rr[:], float(a_val))
                bp = stat.tile([P, 1], F32, name=f"bp{j}", tag="bp")
                nc.vector.tensor_tensor(bp[:], mv[:, 0:1], rr[:], OP.mult)
                nc.vector.tensor_scalar(bp[:], bp[:], -1.0, float(b_val),
                                        OP.mult, OP.add)
                h = tmp.tile([P, D], BF16, name=f"h{j}", tag="h")
                nc.scalar.activation(h[:], src_big[:, j, :], AF.Identity,
                                     bias=bp[:], scale=rr[:])
                if burst_pool is not None and j in (0, 2):
                    heat_burst(burst_pool, 6, f"ln{j}")
                for half in range(2):
                    tp = tp_psum.tile([P, 512], BF16, name=f"tp{j}_{half}",
                                      tag="tp")
                    for k in range(4):
                        cc = half * 4 + k
                        nc.tensor.transpose(tp[:, k * P:(k + 1) * P],
                                            h[:, cc * P:(cc + 1) * P],
                                            identb[:])
                    nc.vector.tensor_copy(
                        hT[:, half * 4:(half + 1) * 4, j * P:(j + 1) * P],
                        tp.rearrange("p (c q) -> p c q", q=P))

        groups = [list(range(NCORES))]
        # tiny dummy AllGather issued first: absorbs the ~50us one-time
        # collective-engine startup latency so the real KV gathers are prompt
        warm_in = dram.tile([256], F8, name="warm_in")
        warm_out = dram.tile([NCORES * 256], F8, name="warm_out",
                             addr_space="Shared")
        warm_src = const.tile([1, 256], F8)
        nc.vector.memset(warm_src[:], 0.0)
        nc.sync.dma_start(warm_in.rearrange("(p n) -> p n", p=1),
                          warm_src[:])
        nc.gpsimd.collective_compute(
            "AllGather", OP.bypass, replica_groups=groups,
            ins=[warm_in.opt()], outs=[warm_out.opt()])

        # staggered chunks: first gathers start early to hide D2D latency
        CHUNKS = [(0,), (1,), (2, 3), (4, 5), (6, 7)]
        CHUNK_OF = {hh: gi for gi, hhs in enumerate(CHUNKS) for hh in hhs}
        CHUNK_OFF = {hh: hhs.index(hh) for hhs in CHUNKS for hh in hhs}
        KVCs = [dram.tile([len(hhs) * CH], F8, name=f"kvc{gi}")
                for gi, hhs in enumerate(CHUNKS)]
        GKVs = [dram.tile([NCORES * len(hhs) * CH], F8, name=f"gkv{gi}",
                          addr_space="Shared") for gi, hhs in enumerate(CHUNKS)]

        # W1/W2 stream pools (bf16): DMAs prefetch during attention.
        w1pool = ctx.enter_context(tc.tile_pool(name="w1pool", bufs=32))
        w2pool = ctx.enter_context(tc.tile_pool(name="w2pool", bufs=28))
        x2_pool = ctx.enter_context(tc.tile_pool(name="x2_pool", bufs=1))
        hb_pool = ctx.enter_context(tc.tile_pool(name="hb_pool", bufs=1))

        with (
            tc.tile_pool(name="x_pool", bufs=1) as x_pool,
            tc.tile_pool(name="ctx_pool", bufs=1) as ctx_pool,
        ):
            x_big = x_pool.tile([P, 4, D], F32)
            with tc.tile_pool(name="qt_pool", bufs=1) as qt_pool:
                # fp8 Q tiles; the passive halves are zero (memset once)
                QTA = qt_pool.tile([P, 8, SL], F8, name="QTA")
                QTB = qt_pool.tile([P, 8, SL], F8, name="QTB")
                nc.vector.memset(QTA[64:128, :, :], 0.0)
                nc.vector.memset(QTB[0:64, :, :], 0.0)

                # ---------------- phase 1: LN1 + transpose ----------------
                with tc.tile_pool(name="hT_pool", bufs=1) as hT_pool:
                    hT = hT_pool.tile([P, 8, SL], F8)
                    with tc.tile_pool(name="tp1", bufs=2, space="PSUM") as tpp:
                        for j in range(4):
                            nsp = 4 if j == 0 else 2
                            for hf in range(nsp):
                                w = D // nsp
                                cs = slice(hf * w, (hf + 1) * w)
                                nc.sync.dma_start(
                                    x_big[:, j, cs],
                                    x_d[j * P:(j + 1) * P, cs])
                        layer_norm_to_T(x_big, ln1_a, ln1_b, hT, tpp,
                                        burst_pool=tpp)
                        heat_burst(tpp, 6, "p2")

                    # ------- phase 2: hh-major K/V/Q + chunked gathers -----
                    with (
                        tc.tile_pool(name="wbig", bufs=13) as wbig,
                        tc.tile_pool(name="kvstage", bufs=2) as kvstage,
                        tc.tile_pool(name="qkps", bufs=2, space="PSUM") as qkps,
                    ):
                        wkt, wvt, wqt = [], [], []
                        for nm, wd, lst in (("wk", wk_d, wkt),
                                            ("wv", wv_d, wvt),
                                            ("wq", wq_d, wqt)):
                            for cp in range(4):
                                w = wbig.tile([P, 2, D], F8, name=f"{nm}{cp}",
                                              tag="wbig")
                                nc.sync.dma_start(
                                    w.rearrange("p a b -> p (a b)"),
                                    wd[cp])
                                lst.append(w)

                        for hh in range(8):
                            hs = slice(hh * P, (hh + 1) * P)
                            # K chunk: [128 dk, 512 keys] fp8
                            ps = qkps.tile([P, SL], F32, name=f"kps{hh}",
                                           tag="qk")
                            for cp in range(4):
                                nc.tensor.matmul(
                                    ps[:], wkt[cp][:, :, hs],
                                    hT[:, 2 * cp:2 * cp + 2, :],
                                    start=(cp == 0), stop=(cp == 3),
                                    perf_mode=DR)
                            kstg = kvstage.tile([P, SL], F8,
                                                name=f"kstg{hh}", tag="kstg")
                            nc.scalar.activation(kstg[:], ps[:], AF.Identity,
                                                 bias=bk_t[:, hh:hh + 1])
                            cb = KVCs[CHUNK_OF[hh]][CHUNK_OFF[hh] * CH:]
                            nc.sync.dma_start(
                                cb[0:KN].rearrange("(d q) -> d q", q=SL),
                                kstg[:])
                            # V chunk: [512 keys, 128 dims] fp8
                            vstg = kvstage.tile([P, 4, P], F8,
                                                name=f"vstg{hh}", tag="vstg")
                            for sb in range(4):
                                psv = qkps.tile([P, P], F32,
                                                name=f"vps{hh}_{sb}", tag="qk")
                                for cp in range(4):
                                    nc.tensor.matmul(
                                        psv[:],
                                        hT[:, 2 * cp:2 * cp + 2,
                                           sb * P:(sb + 1) * P],
                                        wvt[cp][:, :, hs],
                                        start=(cp == 0), stop=False,
                                        perf_mode=DR)
                                nc.tensor.matmul(psv[:], ones1[:], bvr[:, hs],
                                                 start=False, stop=True)
                                nc.scalar.copy(vstg[:, sb, :], psv[:])
                            nc.sync.dma_start(
                                cb[KN:CH].rearrange(
                                    "(sb p e) -> p sb e", p=P, e=P),
                                vstg[:])
                            # Q chunk -> fp8 QTA/QTB active halves
                            psq = qkps.tile([P, SL], F32, name=f"qps{hh}",
                                            tag="qk")
                            for cp in range(4):
                                nc.tensor.matmul(
                                    psq[:], wqt[cp][:, :, hs],
                                    hT[:, 2 * cp:2 * cp + 2, :],
                                    start=(cp == 0), stop=(cp == 3),
                                    perf_mode=DR)
                            nc.scalar.activation(
                                QTA[0:64, hh, :], psq[0:64, :], AF.Identity,
                                bias=bq8[0:64, hh:hh + 1], scale=1.0 / 8.0)
                            nc.scalar.activation(
                                QTB[64:128, hh, :], psq[64:128, :],
                                AF.Identity, bias=bq8[64:128, hh:hh + 1],
                                scale=1.0 / 8.0)
                            gi = CHUNK_OF[hh]
                            if CHUNK_OFF[hh] == len(CHUNKS[gi]) - 1:
                                nc.gpsimd.collective_compute(
                                    "AllGather", OP.bypass,
                                    replica_groups=groups,
                                    ins=[KVCs[gi].opt()],
                                    outs=[GKVs[gi].opt()])

                # Wo (fp8) + W1 + W2 prefetch: DMAs are threaded through the
                # attention steps so they don't jam the KV-gather D2D traffic
                wot = []
                w1t = [[None] * 8 for _ in range(4)]
                w2ts_all = [None] * 32
                prefetch_q = []

                def _pf_wo(cp):
                    w = w1pool.tile([P, 2, D], F8, name=f"wo{cp}", tag="w1")
                    nc.sync.dma_start(w.rearrange("p a b -> p (a b)"),
                                      wo_d[cp])
                    wot.append(w)

                def _pf_w1(qq, cc):
                    w = w1pool.tile([P, F // 4], BF16,
                                    name=f"w1_{qq}_{cc}", tag="w1")
                    nc.sync.dma_start(
                        w[:], w1_d[cc * P:(cc + 1) * P,
                                   qq * 1024:(qq + 1) * 1024])
                    w1t[qq][cc] = w

                def _pf_w2(fg):
                    w2t = w2pool.tile([P, D], BF16, name=f"w2_{fg}",
                                      tag="w2")
                    nc.sync.dma_start(w2t[:], w2_d[fg * P:(fg + 1) * P, :])
                    w2ts_all[fg] = w2t

                for cp in range(4):
                    prefetch_q.append((lambda cp=cp: _pf_wo(cp)))
                for qq in range(2):
                    for cc in range(8):
                        prefetch_q.append((lambda q=qq, c=cc: _pf_w1(q, c)))
                for fg in range(16):
                    prefetch_q.append((lambda f=fg: _pf_w2(f)))
                for qq in range(2, 4):
                    for cc in range(8):
                        prefetch_q.append((lambda q=qq, c=cc: _pf_w1(q, c)))
                for fg in range(16, 32):
                    prefetch_q.append((lambda f=fg: _pf_w2(f)))

                # ---------------- phase 4: attention ----------------
                ctxT = ctx_pool.tile([P, 8, SL], F8)
                with (
                    tc.tile_pool(name="kst", bufs=8) as kst,
                    tc.tile_pool(name="vst", bufs=1) as vst,
                    tc.tile_pool(name="esb", bufs=6) as esb,
                    tc.tile_pool(name="bcs_pool", bufs=2) as bcs_pool,
                    tc.tile_pool(name="rs_pool", bufs=1) as rs_pool,
                    tc.tile_pool(name="spsum", bufs=2, space="PSUM") as spsum,
                    tc.tile_pool(name="cpsum", bufs=2, space="PSUM") as cpsum,
                ):
                    # rotating fp8 V buffers with pre-set ones columns
                    vt_bufs = [vst.tile([P, 4, 2 * P], F8, name=f"vtb{i}")
                               for i in range(4)]
                    for vb in vt_bufs:
                        nc.vector.memset(vb[:], 0.0)
                        nc.vector.memset(vb[:, :, 64], 1.0)
                        nc.vector.memset(vb[:, :, 192], 1.0)

                    cps_all = {}

                    def get_cps(hh, i):
                        key = (hh, i)
                        if key not in cps_all:
                            cps_all[key] = cpsum.tile(
                                [P, SL], F32, name=f"ctx{hh}_{i}",
                                tag=f"ctx{i}")
                        return cps_all[key]

                    kts = {}
                    vt4s = {}

                    def load_kv(hh, c):
                        kt = kst.tile([P, SL], F8, name=f"kt{hh}_{c}",
                                      tag="kt")
                        gi = CHUNK_OF[hh]
                        csz = len(CHUNKS[gi]) * CH
                        base = c * csz + CHUNK_OFF[hh] * CH
                        ksec = GKVs[gi][base: base + KN].rearrange(
                            "(d q) -> d q", q=SL)
                        nc.sync.dma_start(kt[:], ksec)
                        kts[(hh, c)] = kt
                        vb = vt_bufs[(hh * NCORES + c) % 4]
                        vsec = GKVs[gi][base + KN: base + CH].rearrange(
                            "(s e) -> s e", e=P)
                        nc.sync.dma_start(
                            vb[:, :, 0:64],
                            vsec[:, 0:64].rearrange("(kbl p) e -> p kbl e",
                                                    p=P))
                        nc.sync.dma_start(
                            vb[:, :, 128:192],
                            vsec[:, 64:128].rearrange("(kbl p) e -> p kbl e",
                                                      p=P))
                        vt4s[(hh, c)] = vb

                    steps = [(hh, c, h01, g)
                             for hh in range(8)
                             for c in range(NCORES)
                             for h01 in range(2)
                             for g in range(2)]

                    def emit_scores_exp(step, si):
                        hh, c, h01, g = step
                        if (hh, c) not in kts:
                            load_kv(hh, c)
                        kt = kts[(hh, c)]
                        rhs_q = (QTA if h01 == 0 else QTB)[:, hh, :]
                        # separate single-bank psum tiles per half so each is
                        # freed independently by its one exp reader
                        ak = si % 2
                        dk = 1 - ak
                        sps = [None, None]
                        for kk in (dk, ak):
                            sps[kk] = spsum.tile(
                                [P, 512], F32,
                                name=f"sp{hh}_{c}_{h01}_{g}_{kk}",
                                tag=f"sp{kk}")
                            kbl = g * 2 + kk
                            nc.tensor.matmul(
                                sps[kk][:], kt[:, kbl * P:(kbl + 1) * P],
                                rhs_q, start=True, stop=True)
                        et = esb.tile([P, 2, 512], F8,
                                      name=f"e{hh}_{c}_{h01}_{g}", tag="et")
                        # both engines in parallel: DVE bit-trick on the half
                        # computed first, ACT true-exp on the other
                        nc.vector.tensor_scalar(
                            et[:, dk, :].bitcast(I8), sps[dk][:],
                            LOG2E_8, EXPBIAS8, OP.mult, OP.add)
                        nc.scalar.activation(et[:, ak, :], sps[ak][:],
                                             AF.Exp)
                        return (step, et)

                    def emit_ctx(item):
                        (hh, c, h01, g), et = item
                        vb = vt4s[(hh, c)]
                        nc.tensor.matmul(
                            get_cps(hh, h01)[:],
                            vb[:, 2 * g:2 * g + 2, h01 * P:(h01 + 1) * P],
                            et[:],
                            start=(c == 0 and g == 0),
                            stop=(c == 7 and g == 1),
                            perf_mode=DR)

                    bcs_of = {}

                    def emit_norm_a(hh):
                        # reciprocal + broadcast; emitted only once the ctx
                        # psum stop has executed so no engine-queue blocking
                        cps = [cps_all[(hh, 0)], cps_all[(hh, 1)]]
                        den = rs_pool.tile([33, SL], F32, name=f"den{hh}",
                                           tag="den")
                        nc.scalar.copy(den[0:1, :], cps[0][64:65, :])
                        nc.scalar.copy(den[32:33, :], cps[1][64:65, :])
                        # 1/x as exp(-ln x) on ACT: keeps the slow DVE
                        # reciprocal out of the in-order DVE exp stream
                        lnt = rs_pool.tile([33, SL], F32, name=f"lnt{hh}",
                                           tag="lnt")
                        nc.scalar.activation(lnt[:], den[:], AF.Ln)
                        rcf = rs_pool.tile([33, SL], F32, name=f"rcf{hh}",
                                           tag="rcf")
                        nc.scalar.activation(rcf[:], lnt[:], AF.Exp,
                                             scale=-1.0)
                        # partition_broadcast requires in AND out at
                        # partition base 0 -> separate base-0 tiles
                        rcb = rs_pool.tile([1, SL], F32, name=f"rcb{hh}",
                                           tag="rcb")
                        nc.scalar.copy(rcb[:], rcf[32:33, :])
                        bcsa = bcs_pool.tile([64, SL], F32, name=f"bcsa{hh}",
                                             tag="bcsa")
                        bcsb = bcs_pool.tile([64, SL], F32, name=f"bcsb{hh}",
                                             tag="bcsb")
                        nc.gpsimd.partition_broadcast(bcsa[:], rcf[0:1, :])
                        nc.gpsimd.partition_broadcast(bcsb[:], rcb[0:1, :])
                        bcs_of[hh] = (bcsa, bcsb)

                    def emit_norm_b(hh):
                        # the two DVE mults, emitted after broadcasts landed
                        cps = [cps_all.pop((hh, 0)), cps_all.pop((hh, 1))]
                        bcsa, bcsb = bcs_of.pop(hh)
                        nc.vector.tensor_tensor(ctxT[0:64, hh, :],
                                                cps[0][0:64, :],
                                                bcsa[:], OP.mult)
                        nc.vector.tensor_tensor(ctxT[64:128, hh, :],
                                                cps[1][0:64, :],
                                                bcsb[:], OP.mult)

                    norm_qa = []
                    norm_qb = []
                    pend = []
                    for si, step in enumerate(steps):
                        if si + 6 < len(steps):
                            nhh, ncc, _, _ = steps[si + 6]
                            if (nhh, ncc) not in kts:
                                load_kv(nhh, ncc)
                        if si >= 40 and prefetch_q and si % 3 == 0:
                            prefetch_q.pop(0)()
                        pend.append(emit_scores_exp(step, si))
                        if len(pend) > 3:
                            it = pend.pop(0)
                            emit_ctx(it)
                            phh, pc, ph01, pg = it[0]
                            if pc == 7 and ph01 == 1 and pg == 1:
                                norm_qa.append((phh, si + 6))
                        if norm_qa and si >= norm_qa[0][1]:
                            hh_n = norm_qa.pop(0)[0]
                            emit_norm_a(hh_n)
                            norm_qb.append((hh_n, si + 3))
                        if norm_qb and si >= norm_qb[0][1]:
                            emit_norm_b(norm_qb.pop(0)[0])
                    while prefetch_q:
                        prefetch_q.pop(0)()
                    for it in pend:
                        emit_ctx(it)
                        phh, pc, ph01, pg = it[0]
                        if pc == 7 and ph01 == 1 and pg == 1:
                            norm_qa.append((phh, 0))
                    for hh, _ in norm_qa:
                        emit_norm_a(hh)
                        norm_qb.append((hh, 0))
                    for hh, _ in norm_qb:
                        emit_norm_b(hh)

            # ---------------- phase 5: out-proj + residual ----------------
            x2 = x2_pool.tile([P, 4, D], BF16)
            with (
                tc.tile_pool(name="ops", bufs=2, space="PSUM") as opps,
                tc.tile_pool(name="hps5", bufs=1, space="PSUM") as hps5,
            ):
                heat_burst(hps5, 10, "oproj")
                for sb in range(4):
                    for eb in range(2):
                        ps = opps.tile([P, 512], F32, name=f"op{sb}_{eb}",
                                       tag="op")
                        for cp in range(4):
                            nc.tensor.matmul(
                                ps[:],
                                ctxT[:, 2 * cp:2 * cp + 2,
                                     sb * P:(sb + 1) * P],
                                wot[cp][:, :, eb * 512:(eb + 1) * 512],
                                start=(cp == 0), stop=False, perf_mode=DR)
                        nc.tensor.matmul(ps[:], ones65[32:33, :],
                                         bor[:, eb * 512:(eb + 1) * 512],
                                         start=False, stop=True)
                        nc.vector.tensor_tensor(
                            x2[:, sb, eb * 512:(eb + 1) * 512], ps[:],
                            x_big[:, sb, eb * 512:(eb + 1) * 512], OP.add)

        # ---------------- phase 6: LN2 + transpose (bf16) ----------------
        with tc.tile_pool(name="h2T_pool", bufs=1) as h2T_pool:
            h2T = h2T_pool.tile([P, 8, SL], BF16)
            with tc.tile_pool(name="tp2", bufs=2, space="PSUM") as tpp2:
                layer_norm_to_T(x2, ln2_a, ln2_b, h2T, tpp2,
                                burst_pool=tpp2)

            # ------------- phases 7/8: FFN in two halves (bf16) -----------
            with (
                tc.tile_pool(name="atpool", bufs=4) as atpool,
                tc.tile_pool(name="o2ppool", bufs=1) as o2ppool,
                tc.tile_pool(name="outpool", bufs=3) as outpool,
                tc.tile_pool(name="f1ps", bufs=2, space="PSUM") as f1ps,
                tc.tile_pool(name="f2ps", bufs=4, space="PSUM") as f2ps,
            ):
                o2p = o2ppool.tile([P, 4, D], F32)
                with tc.tile_pool(name="hps7", bufs=1, space="PSUM") as hps7:
                    heat_burst(hps7, 10, "ffn")
                for half in range(2):
                    at_h = []
                    for qq in range(half * 2, half * 2 + 2):
                        ATq = atpool.tile([P, 8, SL], BF16,
                                          name=f"at{qq}", tag="at")
                        for fc in range(8):
                            fg = qq * 8 + fc
                            ps = f1ps.tile([P, SL], F32, name=f"f1_{fg}",
                                           tag="f1")
                            for cc in range(8):
                                nc.tensor.matmul(
                                    ps[:],
                                    w1t[qq][cc][:, fc * P:(fc + 1) * P],
                                    h2T[:, cc, :], start=(cc == 0),
                                    stop=(cc == 7))
                            nc.vector.tensor_scalar(ATq[:, fc, :], ps[:],
                                                    b1_t[:, fg:fg + 1],
                                                    0.0, OP.add, OP.max)
                        at_h.append(ATq)
                    w2ts = w2ts_all[half * 16:half * 16 + 16]
                    for eb in range(2):
                        sl = slice(eb * 512, (eb + 1) * 512)
                        pss = [f2ps.tile([P, 512], F32,
                                         name=f"f2_{half}_{eb}_{sb}",
                                         tag="f2") for sb in range(4)]
                        for fcl in range(16):
                            qq, fc = divmod(fcl, 8)
                            for sb in range(4):
                                nc.tensor.matmul(
                                    pss[sb][:],
                                    at_h[qq][:, fc, sb * P:(sb + 1) * P],
                                    w2ts[fcl][:, sl],
                                    start=(fcl == 0),
                                    stop=(half == 0 and fcl == 15))
                        for sb in range(4):
                            ps = pss[sb]
                            if half == 0:
                                nc.vector.tensor_tensor(
                                    o2p[:, sb, sl], ps[:], x2[:, sb, sl],
                                    OP.add)
                            else:
                                nc.tensor.matmul(ps[:], ones65[64:65, :],
                                                 b2r[:, sl],
                                                 start=False, stop=True)
                                ot = outpool.tile([P, 512], F32,
                                                  name=f"ot{sb}_{eb}",
                                                  tag="ot")
                                nc.vector.tensor_tensor(ot[:], ps[:],
                                                        o2p[:, sb, sl],
                                                        OP.add)
                                for dh in range(2):
                                    ds = slice(eb * 512 + dh * 256,
                                               eb * 512 + (dh + 1) * 256)
                                    nc.sync.dma_start(
                                        y_d[sb * P:(sb + 1) * P, ds],
                                        ot[:, dh * 256:(dh + 1) * 256])

    nc.compile()
    return nc


def _stack_pairs(w):
    """[D, N] -> [4, 128, 2*N] with [cp, p, j*N:(j+1)*N] = w[cp*256+j*128+p]."""
    Dd, N = w.shape
    return np.ascontiguousarray(
        w.reshape(4, 2, P, N).transpose(0, 2, 1, 3).reshape(4, P, 2 * N))


def make_in_maps(inp):
    import ml_dtypes
    BF = ml_dtypes.bfloat16
    E4 = ml_dtypes.float8_e4m3
    xf = inp["x"].reshape(S, D)
    shared = {
        "Wq_stk": _stack_pairs(inp["Wq"].astype(E4)),
        "Wk_stk": _stack_pairs(inp["Wk"].astype(E4)),
        "Wv_stk": _stack_pairs(inp["Wv"].astype(E4)),
        "Wo_stk": _stack_pairs(inp["Wo"].astype(E4)),
        "W1": inp["W1"].astype(BF), "W2": inp["W2"].astype(BF),
        "bq": inp["bq"], "bk": inp["bk"], "b1": inp["b1"],
        "bx3": np.stack([inp["bv"], inp["bo"], inp["b2"]]).astype(BF),
    }
    in_maps = []
    for c in range(NCORES):
        m = dict(shared)
        m["x_loc"] = np.ascontiguousarray(xf[c * SL:(c + 1) * SL, :])
        in_maps.append(m)
    return in_maps


def kernel(**inputs):
    inp = {k: np.asarray(v, dtype=np.float32) for k, v in inputs.items()}
    x = inp["x"]
    B = x.shape[0]
    key = (float(inp["ln1_a"][0]), float(inp["ln1_b"][0]),
           float(inp["ln2_a"][0]), float(inp["ln2_b"][0]))
    if key not in _CACHE:
        _CACHE[key] = _build(*key)
    nc = _CACHE[key]

    res = run_bass_kernel_spmd(nc, make_in_maps(inp), list(range(NCORES)))
    out = np.concatenate([res.results[c]["y_loc"] for c in range(NCORES)],
                         axis=0)
    return out.reshape(B, S, D)
